# revision 2
# baseline (speedup 1.0000x reference)
"""Trainium2 Bass kernel for nn_CorrLoss — v2.

loss = mean_i relu( max_{j: t_j != t_i} corr[i,j] - min_{j: t_j == t_i} corr[i,j] + 40 )
corr = feat @ feat.T, feat [4096, 512] f32, targets [4096] int.

Design:
- One-hot class-code contraction extension: lhs +128*onehot(t_i), rhs
  -128*onehot(t_j) => the PE computes corr' = corr - 16384*same directly.
  Then ap = min(corr') + 16384 over a small window, an = max(corr').
- Rows AND columns sorted by class (host-side, loss is row-permutation
  invariant): each 128-row block's positive columns lie in a +-MAR window
  around the block, so the min side is a couple of narrow f32 PSUM reduces
  (exact).  Any superset window is exact thanks to the -16384 offset.
- Max side: Act engine copies each round's 4 PSUM tiles to SBUF as bf16
  (an is O(100), bf16 ulp ~0.5 => ~0.1% loss error), DVE folds a running
  bf16 max at 2x DVE rate.  Final round folds/reduces stay in f32 PSUM.
- Row-data-parallel over 8 cores; per-core column chunks uploaded in
  rotated order (region r = global chunk (c-1+L[r])%8) so one SPMD program
  serves all cores, and the stationary operand is sliced from region 0.
- PE clock-ramp warm-up matmuls run on scratch during the input DMA.
"""
import sys
from contextlib import ExitStack

import numpy as np

sys.path.insert(0, "/opt/trn_rl_repo")

import concourse.bass as bass  # noqa: E402
from concourse import mybir  # noqa: E402
from concourse.bass_utils import run_bass_kernel_spmd  # noqa: E402

N_CORES = 8
N_ROWS = 4096
D = 512
NCLS = 64
M = N_ROWS // N_CORES      # 512 local rows
MT = M // 128              # 4 row tiles
NCHUNK = 512
NT = N_ROWS // NCHUNK      # 8 column chunks
KT = D // 128              # 4 feature k-steps
KE = KT + 1                # + one-hot extension step
SLOTW = KE * NCHUNK        # 2560 cols per chunk region in SBUF
MARGIN = 40.0
BIG = 16384.0              # 128 * 128, exact in bf16/f32
CODE = 128.0
N_WARMUP = 48              # PE clock-ramp warm-up matmuls during input DMA
W_WARMUP = 64
# region r holds global chunk (c - 1 + L[r]) % 8; region 0 = own chunk
L = [1, 0, 2, 3, 4, 5, 6, 7]
POS_OF_SLOT = {1: 0, 0: 1, 2: 2}   # slot -> PE round position (for windows)

_CACHE = {}


def _window_pieces(mar):
    """Min-window pieces: (m, tile_c, col_lo, col_hi) in tile coordinates.

    Window for row-block m covers slot-space cols
    [512 + 128m - mar, 512 + 128m + 128 + mar), slots numbered with the
    core's own chunk at slot 1 (region 0).
    """
    pieces = []
    for m in range(MT):
        lo = 512 + 128 * m - mar
        hi = 512 + 128 * m + 128 + mar
        for s in (0, 1, 2):
            a = max(lo, 512 * s)
            b = min(hi, 512 * s + 512)
            if a < b:
                c = 4 * POS_OF_SLOT[s] + m
                pieces.append((m, c, a - 512 * s, b - 512 * s))
    pieces.sort(key=lambda p: p[1])
    return pieces


def _build(mar):
    f32 = mybir.dt.float32
    bf16 = mybir.dt.bfloat16
    op = mybir.AluOpType
    ax = mybir.AxisListType
    FMAX = 3.4e38
    pieces = _window_pieces(mar)
    n_pieces = len(pieces)
    # pieces that read tiles of round r must be done before round r+2
    def wsem_need(r):
        return sum(1 for (_m, c, _a, _b) in pieces if c < 4 * (r - 1))

    nc = bass.Bass("TRN2", target_bir_lowering=False, debug=False)
    # host layout: [lx (512) | region0 (2560) | ... | region7 (2560)]
    rx = nc.declare_dram_parameter("rx", [128, NCHUNK + NT * SLOTW], bf16,
                                   isOutput=False)
    pl = nc.declare_dram_parameter("pl", [128, MT], f32, isOutput=True)

    def bank(c):
        return ((c // 4) % 2) * 4 + (c % 4)

    with ExitStack() as ctx:
        rxs = ctx.enter_context(
            nc.sbuf_tensor("rxs", [128, NCHUNK + NT * SLOTW], bf16))
        wup = ctx.enter_context(nc.sbuf_tensor("wup", [128, 128], bf16))
        cbuf = [ctx.enter_context(nc.sbuf_tensor(f"cbuf{i}", [128, 2048], bf16))
                for i in range(2)]
        rmx = ctx.enter_context(nc.sbuf_tensor("rmx", [128, 2048], bf16))
        rw = ctx.enter_context(nc.sbuf_tensor("rw", [128, n_pieces], f32))
        an6 = ctx.enter_context(nc.sbuf_tensor("an6", [128, MT], f32))
        t7 = ctx.enter_context(nc.sbuf_tensor("t7", [128, MT], f32))
        apv = ctx.enter_context(nc.sbuf_tensor("apv", [128, MT], f32))
        anv = ctx.enter_context(nc.sbuf_tensor("anv", [128, MT], f32))
        pl_sb = ctx.enter_context(nc.sbuf_tensor("pl_sb", [128, MT], f32))
        ptall = ctx.enter_context(nc.psum_tensor("ptall", [128, 4096], f32))
        dsem = [ctx.enter_context(nc.semaphore(f"dsem{r}")) for r in range(NT)]
        dout = ctx.enter_context(nc.semaphore("dout"))
        mm_sem = ctx.enter_context(nc.semaphore("mm_sem"))
        asem = ctx.enter_context(nc.semaphore("asem"))
        fsem = ctx.enter_context(nc.semaphore("fsem"))
        wsem = ctx.enter_context(nc.semaphore("wsem"))
        isem = ctx.enter_context(nc.semaphore("isem"))
        done_sem = ctx.enter_context(nc.semaphore("done_sem"))
        block = ctx.enter_context(nc.Block())

        @block.gpsimd
        def _(gpsimd):
            nc.gpsimd.memset(wup[:], 0.0).then_inc(isem, 1)

        @block.sync
        def _(sync):
            # first DMA: lx + region0 (own chunk: stationary + first moving)
            sync.dma_start(rxs[:, 0:NCHUNK + SLOTW],
                           rx[:, 0:NCHUNK + SLOTW]).then_inc(dsem[0], 16)
            for r in range(1, NT):
                o = NCHUNK + r * SLOTW
                sync.dma_start(rxs[:, o:o + SLOTW],
                               rx[:, o:o + SLOTW]).then_inc(dsem[r], 16)
            sync.wait_ge(done_sem, 1)
            sync.dma_start(pl[:], pl_sb[:]).then_inc(dout, 16)
            sync.wait_ge(dout, 16)

        @block.tensor
        def _(tensor):
            tensor.wait_ge(isem, 1)
            for _ in range(N_WARMUP):
                nc.tensor.matmul(ptall[:, 3584:3584 + W_WARMUP], wup[:],
                                 wup[:, 0:W_WARMUP], start=True, stop=True)
            for r in range(NT):
                tensor.wait_ge(dsem[r], 16)
                if r >= 2:
                    # banks reused from round r-2: Act copy + window reads
                    tensor.wait_ge(asem, r - 1)
                    w = wsem_need(r)
                    if w > 0:
                        tensor.wait_ge(wsem, w)
                for m in range(MT):
                    c = r * MT + m
                    b = bank(c)
                    so = NCHUNK + r * SLOTW
                    for k in range(KT):
                        nc.tensor.matmul(
                            ptall[:, b * 512:(b + 1) * 512],
                            rxs[:, NCHUNK + k * NCHUNK + m * 128:
                                NCHUNK + k * NCHUNK + (m + 1) * 128],
                            rxs[:, so + k * NCHUNK:so + (k + 1) * NCHUNK],
                            start=(k == 0), stop=False)
                    mm = nc.tensor.matmul(
                        ptall[:, b * 512:(b + 1) * 512],
                        rxs[:, m * 128:(m + 1) * 128],
                        rxs[:, so + KT * NCHUNK:so + KE * NCHUNK],
                        start=False, stop=True)
                    mm.then_inc(mm_sem, 1)

        @block.scalar
        def _(scalar):
            # stage each round's PSUM half into SBUF as bf16 for the max side
            for r in range(NT - 2):
                scalar.wait_ge(mm_sem, 4 * (r + 1))
                if r >= 2:
                    scalar.wait_ge(fsem, r - 1)
                h = (r % 2) * 2048
                nc.scalar.copy(cbuf[r % 2][:],
                               ptall[:, h:h + 2048]).then_inc(asem, 1)
            # round 6 staged per tile so the fold/reduce chain starts early
            for m in range(MT):
                scalar.wait_ge(mm_sem, 4 * (NT - 2) + m + 1)
                if m == 0:
                    scalar.wait_ge(fsem, NT - 3)
                nc.scalar.copy(cbuf[0][:, 512 * m:512 * (m + 1)],
                               ptall[:, 512 * m:512 * (m + 1)]).then_inc(asem, 1)

        @block.vector
        def _(vector):
            nc.vector.memset(rmx[:], -FMAX)
            # ---- min side: narrow exact f32 window reduces from PSUM ----
            emitted = []
            def emit_piece(i):
                m, c, a, b = pieces[i]
                vector.wait_ge(mm_sem, c + 1)
                bk = bank(c)
                nc.vector.tensor_reduce(
                    rw[:, i:i + 1], ptall[:, bk * 512 + a:bk * 512 + b],
                    axis=ax.X, op=op.min).then_inc(wsem, 1)
            # pieces available in rounds 0-1 first
            i = 0
            while i < n_pieces and pieces[i][1] < 8:
                emit_piece(i)
                i += 1
            # ---- max side: fold bf16 copies, interleave remaining pieces --
            for r in range(NT - 2):
                while i < n_pieces and pieces[i][1] + 1 <= 4 * r + 4:
                    emit_piece(i)
                    i += 1
                vector.wait_ge(asem, r + 1)
                nc.vector.tensor_tensor(
                    rmx[:], cbuf[r % 2][:], rmx[:], op=op.max).then_inc(fsem, 1)
            while i < n_pieces:
                emit_piece(i)
                i += 1
            # combine the window pieces into apv[:, m]; drain first so the
            # just-written rw columns are visible (same-engine RAW hazard)
            nc.vector.drain()
            for m in range(MT):
                idx = [j for j, p in enumerate(pieces) if p[0] == m]
                if len(idx) == 1:
                    nc.vector.tensor_scalar(
                        apv[:, m:m + 1], rw[:, idx[0]:idx[0] + 1], FMAX, 0.0,
                        op0=op.min)
                else:
                    nc.vector.tensor_tensor(
                        apv[:, m:m + 1], rw[:, idx[0]:idx[0] + 1],
                        rw[:, idx[1]:idx[1] + 1], op=op.min)
            # round 6: per-tile fold + early final reduce of the bf16 max
            for m in range(MT):
                vector.wait_ge(asem, NT - 2 + m + 1)
                nc.vector.tensor_tensor(
                    rmx[:, 512 * m:512 * (m + 1)],
                    cbuf[0][:, 512 * m:512 * (m + 1)],
                    rmx[:, 512 * m:512 * (m + 1)], op=op.max)
                nc.vector.tensor_reduce(
                    an6[:, m:m + 1], rmx[:, 512 * m:512 * (m + 1)],
                    axis=ax.X, op=op.max)
            # round 7: f32 reduces from PSUM, then vectorized combine with
            # drains between dependent steps
            for m in range(MT):
                c = (NT - 1) * MT + m
                vector.wait_ge(mm_sem, c + 1)
                bk = bank(c)
                nc.vector.tensor_reduce(
                    t7[:, m:m + 1], ptall[:, bk * 512:(bk + 1) * 512],
                    axis=ax.X, op=op.max)
            nc.vector.drain()
            nc.vector.tensor_tensor(anv[:], t7[:], an6[:], op=op.max)
            nc.vector.drain()
            nc.vector.tensor_tensor(anv[:], anv[:], apv[:], op=op.subtract)
            nc.vector.drain()
            fin = nc.vector.tensor_scalar(
                pl_sb[:], anv[:], MARGIN - BIG, 0.0, op0=op.add, op1=op.max)
            fin.then_inc(done_sem, 1)
    return nc


def _prep(feat: np.ndarray, targets: np.ndarray):
    import ml_dtypes
    bf16 = ml_dtypes.bfloat16
    feat = np.asarray(feat, dtype=np.float32)
    tg = np.asarray(targets).astype(np.int64)

    perm = np.argsort(tg, kind="stable")
    tgs = tg[perm]
    feats = feat[perm]

    counts = np.bincount(tgs, minlength=NCLS)
    mar = 96
    maxc = int(counts.max())
    if maxc - 1 > mar:
        mar = int(np.ceil((maxc - 1) / 32) * 32)
    assert mar <= 384, "class sizes too skewed for the window scheme"

    featx = np.zeros((KE * 128, N_ROWS), dtype=bf16)
    featx[:D, :] = feats.T.astype(bf16)
    featx[D + tgs, np.arange(N_ROWS)] = bf16(-CODE)

    in_maps = []
    for c in range(N_CORES):
        rxa = np.empty((128, NCHUNK + NT * SLOTW), dtype=bf16)
        tloc = tgs[c * M:(c + 1) * M]
        lxa = np.zeros((128, NCHUNK), dtype=bf16)
        lxa[tloc, np.arange(M)] = bf16(CODE)
        rxa[:, :NCHUNK] = lxa
        for r in range(NT):
            gc = (c - 1 + L[r]) % NT
            blk = featx[:, gc * NCHUNK:(gc + 1) * NCHUNK]  # [640, 512]
            rxa[:, NCHUNK + r * SLOTW:NCHUNK + (r + 1) * SLOTW] = (
                blk.reshape(KE, 128, NCHUNK).transpose(1, 0, 2).reshape(128, SLOTW))
        in_maps.append({"rx": rxa})
    return in_maps, mar


def kernel(feat: np.ndarray, targets: np.ndarray) -> np.ndarray:
    in_maps, mar = _prep(feat, targets)
    key = ("nc", mar)
    if key not in _CACHE:
        _CACHE[key] = _build(mar)
    nc = _CACHE[key]
    res = run_bass_kernel_spmd(nc, in_maps, list(range(N_CORES)))
    total = 0.0
    for c in range(N_CORES):
        total += res.results[c]["pl"].astype(np.float64).sum()
    return np.asarray(np.float32(total / N_ROWS))


# revision 5
# speedup vs baseline: 1.7639x; 1.7639x over previous
"""Trainium2 Bass kernel for nn_CorrLoss — v5.

fp8e4 DoubleRow Gram matmuls (3 k-pair steps per [128,512] tile) with the
one-hot class-code contraction extension (corr' = corr - 16384*same).
Rows+columns class-sorted on host: the min-over-positives side is a few
narrow exact f32 PSUM window reduces; the max-over-negatives side is a
running elementwise max: per round, tile m3 folds straight from PSUM f32
on DVE while tiles m0-2 are staged to SBUF as bf16 by the Act engine and
folded at 2x DVE rate.  One [128,4,512] reduce finishes the max side.
Row-data-parallel across 8 cores via rotated column-chunk upload.
"""
import sys
from contextlib import ExitStack

import numpy as np

sys.path.insert(0, "/opt/trn_rl_repo")

import concourse.bass as bass  # noqa: E402
from concourse import mybir  # noqa: E402
from concourse.bass_utils import run_bass_kernel_spmd  # noqa: E402

N_CORES = 8
N_ROWS = 4096
D = 512
NCLS = 64
M = N_ROWS // N_CORES      # 512 local rows
MT = M // 128              # 4 row tiles
NCHUNK = 512
NT = N_ROWS // NCHUNK      # 8 column chunks
KE = 6                     # 4 feature k-blocks + one-hot ext + zero pad
KP = KE // 2               # 3 DoubleRow pairs
SLOTW = KE * NCHUNK        # 3072
MARGIN = 40.0
BIG = 16384.0
CODE = 128.0
N_WARMUP = 48
W_WARMUP = 64
L = [1, 0, 2, 3, 4, 5, 6, 7]
POS_OF_SLOT = {1: 0, 0: 1, 2: 2}

_CACHE = {}


def _window_pieces(mar):
    pieces = []
    for m in range(MT):
        lo = 512 + 128 * m - mar
        hi = 512 + 128 * m + 128 + mar
        for s in (0, 1, 2):
            a = max(lo, 512 * s)
            b = min(hi, 512 * s + 512)
            if a < b:
                c = 4 * POS_OF_SLOT[s] + m
                pieces.append((m, c, a - 512 * s, b - 512 * s))
    pieces.sort(key=lambda p: p[1])
    return pieces


def _build(mar):
    f32 = mybir.dt.float32
    bf16 = mybir.dt.bfloat16
    fp8 = mybir.dt.float8e4
    op = mybir.AluOpType
    ax = mybir.AxisListType
    DR = mybir.MatmulPerfMode.DoubleRow
    FMAX = 3.4e38
    pieces = _window_pieces(mar)
    n_pieces = len(pieces)

    def wsem_need(r):
        return sum(1 for (_m, c, _a, _b) in pieces if c < 4 * (r - 1))

    nc = bass.Bass("TRN2", target_bir_lowering=False, debug=False)
    # host layout: [lx(ext,pad: 1024) | region0 (3072) | ... | region7]
    rx = nc.declare_dram_parameter("rx", [128, 2 * NCHUNK + NT * SLOTW], fp8,
                                   isOutput=False)
    pl = nc.declare_dram_parameter("pl", [128, 2 * MT], f32, isOutput=True)

    def bank(c):
        return ((c // 4) % 2) * 4 + (c % 4)

    with ExitStack() as ctx:
        # dim1 k-blocks: 0-1 = lx (+code ext, zero pad); region r at 2+6r
        rx3 = ctx.enter_context(
            nc.sbuf_tensor("rx3", [128, 2 + NT * KE, NCHUNK], fp8))
        wup = ctx.enter_context(nc.sbuf_tensor("wup", [128, 128], fp8))
        cbuf = [ctx.enter_context(nc.sbuf_tensor(f"cbuf{i}", [128, 3, NCHUNK], bf16))
                for i in range(2)]
        rmx = ctx.enter_context(nc.sbuf_tensor("rmx", [128, MT, NCHUNK], bf16))
        rw = ctx.enter_context(nc.sbuf_tensor("rw", [128, n_pieces], f32))
        an6 = ctx.enter_context(nc.sbuf_tensor("an6", [128, MT], f32))
        apv = ctx.enter_context(nc.sbuf_tensor("apv", [128, MT], f32))
        anv = ctx.enter_context(nc.sbuf_tensor("anv", [128, MT], f32))
        pl_sb = ctx.enter_context(nc.sbuf_tensor("pl_sb", [128, 2 * MT], f32))
        ptall = ctx.enter_context(nc.psum_tensor("ptall", [128, 8, NCHUNK], f32))
        dsem = [ctx.enter_context(nc.semaphore(f"dsem{r}")) for r in range(NT)]
        dout = ctx.enter_context(nc.semaphore("dout"))
        mm_sem = ctx.enter_context(nc.semaphore("mm_sem"))
        asem = ctx.enter_context(nc.semaphore("asem"))
        fsem = ctx.enter_context(nc.semaphore("fsem"))
        wsem = ctx.enter_context(nc.semaphore("wsem"))
        isem = ctx.enter_context(nc.semaphore("isem"))
        done_sem = ctx.enter_context(nc.semaphore("done_sem"))
        block = ctx.enter_context(nc.Block())

        @block.gpsimd
        def _(gpsimd):
            nc.gpsimd.memset(wup[:], 0.0).then_inc(isem, 1)
            nc.gpsimd.memset(rmx[:], -FMAX).then_inc(isem, 1)

        @block.sync
        def _(sync):
            # first DMA: lx + region0 together
            sync.dma_start(rx3[:, 0:2 + KE, :],
                           rx[:, 0:(2 + KE) * NCHUNK]).then_inc(dsem[0], 16)
            for r in range(1, NT):
                o = 2 * NCHUNK + r * SLOTW
                sync.dma_start(rx3[:, 2 + r * KE:2 + (r + 1) * KE, :],
                               rx[:, o:o + SLOTW]).then_inc(dsem[r], 16)
            sync.wait_ge(done_sem, 1)
            sync.dma_start(pl[:], pl_sb[:]).then_inc(dout, 16)
            sync.wait_ge(dout, 16)

        @block.tensor
        def _(tensor):
            tensor.wait_ge(isem, 1)
            for _ in range(N_WARMUP):
                nc.tensor.matmul(ptall[:, 7, 0:W_WARMUP], wup[:],
                                 wup[:, 0:W_WARMUP], start=True, stop=True)
            for r in range(NT):
                tensor.wait_ge(dsem[r], 16)
                if r >= 2:
                    # round r-2 bank consumers: Act copy, DVE direct fold,
                    # and (rounds 0-2) the min-window reads
                    tensor.wait_ge(asem, r - 1)
                    tensor.wait_ge(fsem, 2 * r - 3)
                    w = wsem_need(r)
                    if w > 0:
                        tensor.wait_ge(wsem, w)
                for m in range(MT):
                    c = r * MT + m
                    b = bank(c)
                    out = ptall[:, b, :]
                    ro = 2 + r * KE
                    for kp in range(KP - 1):
                        nc.tensor.matmul(
                            out,
                            rx3[:, 2 + 2 * kp:2 + 2 * kp + 2,
                                m * 128:(m + 1) * 128],
                            rx3[:, ro + 2 * kp:ro + 2 * kp + 2, :],
                            perf_mode=DR, start=(kp == 0), stop=False)
                    mm = nc.tensor.matmul(
                        out,
                        rx3[:, 0:2, m * 128:(m + 1) * 128],
                        rx3[:, ro + 4:ro + 6, :],
                        perf_mode=DR, start=False, stop=True)
                    mm.then_inc(mm_sem, 1)

        @block.scalar
        def _(scalar):
            # stage tiles m0-2 of each round into SBUF bf16 for the DVE fold
            for r in range(NT):
                scalar.wait_ge(mm_sem, 4 * r + 3)
                if r >= 2:
                    scalar.wait_ge(fsem, 2 * (r - 2) + 2)
                h = (r % 2) * 4
                nc.scalar.copy(cbuf[r % 2][:],
                               ptall[:, h:h + 3, :]).then_inc(asem, 1)

        @block.vector
        def _(vector):
            vector.wait_ge(isem, 2)
            def emit_piece(i):
                m, c, a, b = pieces[i]
                vector.wait_ge(mm_sem, c + 1)
                bk = bank(c)
                nc.vector.tensor_reduce(
                    rw[:, i:i + 1], ptall[:, bk, a:b],
                    axis=ax.X, op=op.min).then_inc(wsem, 1)
            i = 0
            while i < n_pieces and pieces[i][1] < 8:
                emit_piece(i)
                i += 1
            for r in range(NT - 1):
                while i < n_pieces and pieces[i][1] + 1 <= 4 * r + 4:
                    emit_piece(i)
                    i += 1
                # direct f32 fold of tile m3 (gated only on the PE)
                vector.wait_ge(mm_sem, 4 * r + 4)
                h = (r % 2) * 4
                nc.vector.tensor_tensor(
                    rmx[:, MT - 1, :], ptall[:, h + 3, :],
                    rmx[:, MT - 1, :], op=op.max).then_inc(fsem, 1)
                # bf16 fold of the staged m0-2 tiles (2x DVE rate)
                vector.wait_ge(asem, r + 1)
                nc.vector.tensor_tensor(
                    rmx[:, 0:MT - 1, :], cbuf[r % 2][:], rmx[:, 0:MT - 1, :],
                    op=op.max).then_inc(fsem, 1)
            # min-side combines (fills the pipeline drain gap)
            nc.vector.drain()
            for m in range(MT):
                idx = [j for j, p in enumerate(pieces) if p[0] == m]
                if len(idx) == 1:
                    nc.vector.tensor_scalar(
                        pl_sb[:, MT + m:MT + m + 1],
                        rw[:, idx[0]:idx[0] + 1], FMAX, 0.0, op0=op.min)
                else:
                    nc.vector.tensor_tensor(
                        pl_sb[:, MT + m:MT + m + 1], rw[:, idx[0]:idx[0] + 1],
                        rw[:, idx[1]:idx[1] + 1], op=op.min)
            # round 7: fold m3 directly, reduce its row early, then fold the
            # staged m0-2 and finish their reduce — shortens the tail chain
            rl = NT - 1
            vector.wait_ge(mm_sem, 4 * rl + 4)
            h = (rl % 2) * 4
            nc.vector.tensor_tensor(
                rmx[:, MT - 1, :], ptall[:, h + 3, :],
                rmx[:, MT - 1, :], op=op.max)
            nc.vector.tensor_reduce(
                pl_sb[:, MT - 1:MT], rmx[:, MT - 1, :], axis=ax.X, op=op.max)
            vector.wait_ge(asem, rl + 1)
            nc.vector.tensor_tensor(
                rmx[:, 0:MT - 1, :], cbuf[rl % 2][:], rmx[:, 0:MT - 1, :],
                op=op.max)
            fin = nc.vector.tensor_reduce(
                pl_sb[:, 0:MT - 1], rmx[:, 0:MT - 1, :], axis=ax.X, op=op.max)
            fin.then_inc(done_sem, 1)
    return nc


def _prep(feat: np.ndarray, targets: np.ndarray):
    import ml_dtypes
    fp8 = ml_dtypes.float8_e4m3
    feat = np.asarray(feat, dtype=np.float32)
    tg = np.asarray(targets).astype(np.int64)

    perm = np.argsort(tg, kind="stable")
    tgs = tg[perm]
    feats = feat[perm]

    counts = np.bincount(tgs, minlength=NCLS)
    mar = 96
    maxc = int(counts.max())
    if maxc - 1 > mar:
        mar = int(np.ceil((maxc - 1) / 32) * 32)
    assert mar <= 384, "class sizes too skewed for the window scheme"

    featx = np.zeros((KE * 128, N_ROWS), dtype=fp8)
    featx[:D, :] = feats.T.astype(fp8)
    featx[D + tgs, np.arange(N_ROWS)] = fp8(-CODE)

    in_maps = []
    for c in range(N_CORES):
        rxa = np.empty((128, 2 * NCHUNK + NT * SLOTW), dtype=fp8)
        tloc = tgs[c * M:(c + 1) * M]
        lxa = np.zeros((128, 2 * NCHUNK), dtype=fp8)
        lxa[tloc, np.arange(M)] = fp8(CODE)
        rxa[:, :2 * NCHUNK] = lxa
        for r in range(NT):
            gc = (c - 1 + L[r]) % NT
            blk = featx[:, gc * NCHUNK:(gc + 1) * NCHUNK]
            rxa[:, 2 * NCHUNK + r * SLOTW:2 * NCHUNK + (r + 1) * SLOTW] = (
                blk.reshape(KE, 128, NCHUNK).transpose(1, 0, 2).reshape(128, SLOTW))
        in_maps.append({"rx": rxa})
    return in_maps, mar


def kernel(feat: np.ndarray, targets: np.ndarray) -> np.ndarray:
    in_maps, mar = _prep(feat, targets)
    key = ("nc", mar)
    if key not in _CACHE:
        _CACHE[key] = _build(mar)
    nc = _CACHE[key]
    res = run_bass_kernel_spmd(nc, in_maps, list(range(N_CORES)))
    total = 0.0
    for c in range(N_CORES):
        out = res.results[c]["pl"].astype(np.float64)
        an = out[:, :MT]
        ap = out[:, MT:]
        total += np.maximum(an - ap + MARGIN - BIG, 0.0).sum()
    return np.asarray(np.float32(total / N_ROWS))


# revision 6
# speedup vs baseline: 1.7993x; 1.0201x over previous
"""Trainium2 Bass kernel for nn_CorrLoss — v5.

fp8e4 DoubleRow Gram matmuls (3 k-pair steps per [128,512] tile) with the
one-hot class-code contraction extension (corr' = corr - 16384*same).
Rows+columns class-sorted on host: the min-over-positives side is a few
narrow exact f32 PSUM window reduces; the max-over-negatives side is a
running elementwise max: per round, tile m3 folds straight from PSUM f32
on DVE while tiles m0-2 are staged to SBUF as bf16 by the Act engine and
folded at 2x DVE rate.  One [128,4,512] reduce finishes the max side.
Row-data-parallel across 8 cores via rotated column-chunk upload.
"""
import sys
from contextlib import ExitStack

import numpy as np

sys.path.insert(0, "/opt/trn_rl_repo")

import concourse.bass as bass  # noqa: E402
from concourse import mybir  # noqa: E402
from concourse.bass_utils import run_bass_kernel_spmd  # noqa: E402

N_CORES = 8
N_ROWS = 4096
D = 512
NCLS = 64
M = N_ROWS // N_CORES      # 512 local rows
MT = M // 128              # 4 row tiles
NCHUNK = 512
NT = N_ROWS // NCHUNK      # 8 column chunks
KE = 6                     # 4 feature k-blocks + one-hot ext + zero pad
KP = KE // 2               # 3 DoubleRow pairs
SLOTW = KE * NCHUNK        # 3072
MARGIN = 40.0
BIG = 16384.0
CODE = 128.0
N_WARMUP = 48
W_WARMUP = 64
L = [1, 0, 2, 3, 4, 5, 6, 7]
POS_OF_SLOT = {1: 0, 0: 1, 2: 2}

_CACHE = {}


def _window_pieces(mar):
    pieces = []
    for m in range(MT):
        lo = 512 + 128 * m - mar
        hi = 512 + 128 * m + 128 + mar
        for s in (0, 1, 2):
            a = max(lo, 512 * s)
            b = min(hi, 512 * s + 512)
            if a < b:
                c = 4 * POS_OF_SLOT[s] + m
                pieces.append((m, c, a - 512 * s, b - 512 * s))
    pieces.sort(key=lambda p: p[1])
    return pieces


def _build(mar):
    f32 = mybir.dt.float32
    bf16 = mybir.dt.bfloat16
    fp8 = mybir.dt.float8e4
    op = mybir.AluOpType
    ax = mybir.AxisListType
    DR = mybir.MatmulPerfMode.DoubleRow
    FMAX = 3.4e38
    pieces = _window_pieces(mar)
    n_pieces = len(pieces)

    def wsem_need(r):
        return sum(1 for (_m, c, _a, _b) in pieces if c < 4 * (r - 1))

    nc = bass.Bass("TRN2", target_bir_lowering=False, debug=False)
    # host layout: [lx(ext,pad: 1024) | region0 (3072) | ... | region7]
    rx = nc.declare_dram_parameter("rx", [128, 2 * NCHUNK + NT * SLOTW], fp8,
                                   isOutput=False)
    pl = nc.declare_dram_parameter("pl", [128, 2 * MT], f32, isOutput=True)

    def bank(c):
        return ((c // 4) % 2) * 4 + (c % 4)

    with ExitStack() as ctx:
        # dim1 k-blocks: 0-1 = lx (+code ext, zero pad); region r at 2+6r
        rx3 = ctx.enter_context(
            nc.sbuf_tensor("rx3", [128, 2 + NT * KE, NCHUNK], fp8))
        wup = ctx.enter_context(nc.sbuf_tensor("wup", [128, 128], fp8))
        cbuf = [ctx.enter_context(nc.sbuf_tensor(f"cbuf{i}", [128, 3, NCHUNK], bf16))
                for i in range(2)]
        rmx = ctx.enter_context(nc.sbuf_tensor("rmx", [128, MT, NCHUNK], bf16))
        rw = ctx.enter_context(nc.sbuf_tensor("rw", [128, n_pieces], f32))
        an6 = ctx.enter_context(nc.sbuf_tensor("an6", [128, MT], f32))
        apv = ctx.enter_context(nc.sbuf_tensor("apv", [128, MT], f32))
        anv = ctx.enter_context(nc.sbuf_tensor("anv", [128, MT], f32))
        pl_sb = ctx.enter_context(nc.sbuf_tensor("pl_sb", [128, 2 * MT], f32))
        ptall = ctx.enter_context(nc.psum_tensor("ptall", [128, 8, NCHUNK], f32))
        dsem = [ctx.enter_context(nc.semaphore(f"dsem{r}")) for r in range(NT)]
        dout = ctx.enter_context(nc.semaphore("dout"))
        mm_sem = ctx.enter_context(nc.semaphore("mm_sem"))
        asem = ctx.enter_context(nc.semaphore("asem"))
        fsem = ctx.enter_context(nc.semaphore("fsem"))
        wsem = ctx.enter_context(nc.semaphore("wsem"))
        isem = ctx.enter_context(nc.semaphore("isem"))
        done_sem = ctx.enter_context(nc.semaphore("done_sem"))
        block = ctx.enter_context(nc.Block())

        @block.gpsimd
        def _(gpsimd):
            nc.gpsimd.memset(wup[:], 0.0).then_inc(isem, 1)
            nc.gpsimd.memset(rmx[:], -FMAX).then_inc(isem, 1)

        @block.sync
        def _(sync):
            # first DMA: lx + region0 together
            sync.dma_start(rx3[:, 0:2 + KE, :],
                           rx[:, 0:(2 + KE) * NCHUNK]).then_inc(dsem[0], 16)
            for r in range(1, NT):
                o = 2 * NCHUNK + r * SLOTW
                sync.dma_start(rx3[:, 2 + r * KE:2 + (r + 1) * KE, :],
                               rx[:, o:o + SLOTW]).then_inc(dsem[r], 16)
            sync.wait_ge(done_sem, 1)
            sync.dma_start(pl[:], pl_sb[:]).then_inc(dout, 16)
            sync.wait_ge(dout, 16)

        @block.tensor
        def _(tensor):
            tensor.wait_ge(isem, 1)
            for _ in range(N_WARMUP):
                nc.tensor.matmul(ptall[:, 7, 0:W_WARMUP], wup[:],
                                 wup[:, 0:W_WARMUP], start=True, stop=True)
            for r in range(NT):
                tensor.wait_ge(dsem[r], 16)
                for m in range(MT):
                    c = r * MT + m
                    b = bank(c)
                    if r >= 2:
                        # this tile's bank consumer from round r-2: the Act
                        # copy (m0-2) or the DVE direct fold (m3), plus any
                        # min-window reads of that tile
                        if m == 0:
                            tensor.wait_ge(asem, r - 1)
                        if m == MT - 1:
                            tensor.wait_ge(fsem, 2 * r - 3)
                        w = sum(1 for (_pm, pc, _a, _b) in pieces
                                if pc <= 4 * (r - 2) + m)
                        if w > 0:
                            tensor.wait_ge(wsem, w)
                    out = ptall[:, b, :]
                    ro = 2 + r * KE
                    for kp in range(KP - 1):
                        nc.tensor.matmul(
                            out,
                            rx3[:, 2 + 2 * kp:2 + 2 * kp + 2,
                                m * 128:(m + 1) * 128],
                            rx3[:, ro + 2 * kp:ro + 2 * kp + 2, :],
                            perf_mode=DR, start=(kp == 0), stop=False)
                    mm = nc.tensor.matmul(
                        out,
                        rx3[:, 0:2, m * 128:(m + 1) * 128],
                        rx3[:, ro + 4:ro + 6, :],
                        perf_mode=DR, start=False, stop=True)
                    mm.then_inc(mm_sem, 1)

        @block.scalar
        def _(scalar):
            # stage tiles m0-2 of each round into SBUF bf16 for the DVE fold
            for r in range(NT):
                scalar.wait_ge(mm_sem, 4 * r + 3)
                if r >= 2:
                    scalar.wait_ge(fsem, 2 * (r - 2) + 2)
                h = (r % 2) * 4
                nc.scalar.copy(cbuf[r % 2][:],
                               ptall[:, h:h + 3, :]).then_inc(asem, 1)

        @block.vector
        def _(vector):
            vector.wait_ge(isem, 2)
            def emit_piece(i):
                m, c, a, b = pieces[i]
                vector.wait_ge(mm_sem, c + 1)
                bk = bank(c)
                nc.vector.tensor_reduce(
                    rw[:, i:i + 1], ptall[:, bk, a:b],
                    axis=ax.X, op=op.min).then_inc(wsem, 1)
            i = 0
            while i < n_pieces and pieces[i][1] < 8:
                emit_piece(i)
                i += 1
            for r in range(NT - 1):
                while i < n_pieces and pieces[i][1] + 1 <= 4 * r + 4:
                    emit_piece(i)
                    i += 1
                # direct f32 fold of tile m3 (gated only on the PE)
                vector.wait_ge(mm_sem, 4 * r + 4)
                h = (r % 2) * 4
                nc.vector.tensor_tensor(
                    rmx[:, MT - 1, :], ptall[:, h + 3, :],
                    rmx[:, MT - 1, :], op=op.max).then_inc(fsem, 1)
                # bf16 fold of the staged m0-2 tiles (2x DVE rate)
                vector.wait_ge(asem, r + 1)
                nc.vector.tensor_tensor(
                    rmx[:, 0:MT - 1, :], cbuf[r % 2][:], rmx[:, 0:MT - 1, :],
                    op=op.max).then_inc(fsem, 1)
                if r == 3:
                    # min-side combines: all window pieces are in by round 3
                    # and DVE has slack here, keeping them off the tail
                    nc.vector.drain()
                    for m in range(MT):
                        idx = [j for j, p in enumerate(pieces) if p[0] == m]
                        if len(idx) == 1:
                            nc.vector.tensor_scalar(
                                pl_sb[:, MT + m:MT + m + 1],
                                rw[:, idx[0]:idx[0] + 1], FMAX, 0.0,
                                op0=op.min)
                        else:
                            nc.vector.tensor_tensor(
                                pl_sb[:, MT + m:MT + m + 1],
                                rw[:, idx[0]:idx[0] + 1],
                                rw[:, idx[1]:idx[1] + 1], op=op.min)
            # round 7: fold m3 directly, fold the staged m0-2, then one
            # reduce over the whole running max
            rl = NT - 1
            vector.wait_ge(mm_sem, 4 * rl + 4)
            h = (rl % 2) * 4
            nc.vector.tensor_tensor(
                rmx[:, MT - 1, :], ptall[:, h + 3, :],
                rmx[:, MT - 1, :], op=op.max)
            vector.wait_ge(asem, rl + 1)
            nc.vector.tensor_tensor(
                rmx[:, 0:MT - 1, :], cbuf[rl % 2][:], rmx[:, 0:MT - 1, :],
                op=op.max)
            fin = nc.vector.tensor_reduce(
                pl_sb[:, 0:MT], rmx[:], axis=ax.X, op=op.max)
            fin.then_inc(done_sem, 1)
    return nc


def _prep(feat: np.ndarray, targets: np.ndarray):
    import ml_dtypes
    fp8 = ml_dtypes.float8_e4m3
    feat = np.asarray(feat, dtype=np.float32)
    tg = np.asarray(targets).astype(np.int64)

    perm = np.argsort(tg, kind="stable")
    tgs = tg[perm]
    feats = feat[perm]

    counts = np.bincount(tgs, minlength=NCLS)
    # window margin: a block's first/last class can extend at most
    # count-1 columns beyond the block boundary
    mar = max(32, int(counts.max()) - 1)
    assert mar <= 384, "class sizes too skewed for the window scheme"

    featx = np.zeros((KE * 128, N_ROWS), dtype=fp8)
    featx[:D, :] = feats.T.astype(fp8)
    featx[D + tgs, np.arange(N_ROWS)] = fp8(-CODE)

    in_maps = []
    for c in range(N_CORES):
        rxa = np.empty((128, 2 * NCHUNK + NT * SLOTW), dtype=fp8)
        tloc = tgs[c * M:(c + 1) * M]
        lxa = np.zeros((128, 2 * NCHUNK), dtype=fp8)
        lxa[tloc, np.arange(M)] = fp8(CODE)
        rxa[:, :2 * NCHUNK] = lxa
        for r in range(NT):
            gc = (c - 1 + L[r]) % NT
            blk = featx[:, gc * NCHUNK:(gc + 1) * NCHUNK]
            rxa[:, 2 * NCHUNK + r * SLOTW:2 * NCHUNK + (r + 1) * SLOTW] = (
                blk.reshape(KE, 128, NCHUNK).transpose(1, 0, 2).reshape(128, SLOTW))
        in_maps.append({"rx": rxa})
    return in_maps, mar


def kernel(feat: np.ndarray, targets: np.ndarray) -> np.ndarray:
    in_maps, mar = _prep(feat, targets)
    key = ("nc", mar)
    if key not in _CACHE:
        _CACHE[key] = _build(mar)
    nc = _CACHE[key]
    res = run_bass_kernel_spmd(nc, in_maps, list(range(N_CORES)))
    total = 0.0
    for c in range(N_CORES):
        out = res.results[c]["pl"].astype(np.float64)
        an = out[:, :MT]
        ap = out[:, MT:]
        total += np.maximum(an - ap + MARGIN - BIG, 0.0).sum()
    return np.asarray(np.float32(total / N_ROWS))


# revision 7
# speedup vs baseline: 1.8151x; 1.0088x over previous
"""Trainium2 Bass kernel for nn_CorrLoss — v5.

fp8e4 DoubleRow Gram matmuls (3 k-pair steps per [128,512] tile) with the
one-hot class-code contraction extension (corr' = corr - 16384*same).
Rows+columns class-sorted on host: the min-over-positives side is a few
narrow exact f32 PSUM window reduces; the max-over-negatives side is a
running elementwise max: per round, tile m3 folds straight from PSUM f32
on DVE while tiles m0-2 are staged to SBUF as bf16 by the Act engine and
folded at 2x DVE rate.  One [128,4,512] reduce finishes the max side.
Row-data-parallel across 8 cores via rotated column-chunk upload.
"""
import sys
from contextlib import ExitStack

import numpy as np

sys.path.insert(0, "/opt/trn_rl_repo")

import concourse.bass as bass  # noqa: E402
from concourse import mybir  # noqa: E402
from concourse.bass_utils import run_bass_kernel_spmd  # noqa: E402

N_CORES = 8
N_ROWS = 4096
D = 512
NCLS = 64
M = N_ROWS // N_CORES      # 512 local rows
MT = M // 128              # 4 row tiles
NCHUNK = 512
NT = N_ROWS // NCHUNK      # 8 column chunks
KE = 6                     # 4 feature k-blocks + one-hot ext + zero pad
KP = KE // 2               # 3 DoubleRow pairs
SLOTW = KE * NCHUNK        # 3072
MARGIN = 40.0
BIG = 16384.0
CODE = 128.0
N_WARMUP = 48
W_WARMUP = 64
L = [1, 0, 2, 3, 4, 5, 6, 7]
POS_OF_SLOT = {1: 0, 0: 1, 2: 2}

_CACHE = {}


def _window_pieces(mar):
    pieces = []
    for m in range(MT):
        lo = 512 + 128 * m - mar
        hi = 512 + 128 * m + 128 + mar
        for s in (0, 1, 2):
            a = max(lo, 512 * s)
            b = min(hi, 512 * s + 512)
            if a < b:
                c = 4 * POS_OF_SLOT[s] + m
                pieces.append((m, c, a - 512 * s, b - 512 * s))
    pieces.sort(key=lambda p: p[1])
    return pieces


def _build(mar):
    f32 = mybir.dt.float32
    bf16 = mybir.dt.bfloat16
    fp8 = mybir.dt.float8e4
    op = mybir.AluOpType
    ax = mybir.AxisListType
    DR = mybir.MatmulPerfMode.DoubleRow
    FMAX = 3.4e38
    pieces = _window_pieces(mar)
    n_pieces = len(pieces)

    def wsem_need(r):
        return sum(1 for (_m, c, _a, _b) in pieces if c < 4 * (r - 1))

    nc = bass.Bass("TRN2", target_bir_lowering=False, debug=False)
    # host layout: [lx(ext,pad: 1024) | region0 (3072) | ... | region7]
    rx = nc.declare_dram_parameter("rx", [128, 2 * NCHUNK + NT * SLOTW], fp8,
                                   isOutput=False)
    pl = nc.declare_dram_parameter("pl", [128, 2 * MT], f32, isOutput=True)

    def bank(c):
        return ((c // 4) % 2) * 4 + (c % 4)

    with ExitStack() as ctx:
        # dim1 k-blocks: 0-1 = lx (+code ext, zero pad); region r at 2+6r
        rx3 = ctx.enter_context(
            nc.sbuf_tensor("rx3", [128, 2 + NT * KE, NCHUNK], fp8))
        wup = ctx.enter_context(nc.sbuf_tensor("wup", [128, 128], fp8))
        cbuf = [ctx.enter_context(nc.sbuf_tensor(f"cbuf{i}", [128, 3, NCHUNK], bf16))
                for i in range(2)]
        rmx = ctx.enter_context(nc.sbuf_tensor("rmx", [128, MT, NCHUNK], bf16))
        rw = ctx.enter_context(nc.sbuf_tensor("rw", [128, n_pieces], f32))
        an6 = ctx.enter_context(nc.sbuf_tensor("an6", [128, MT], f32))
        apv = ctx.enter_context(nc.sbuf_tensor("apv", [128, MT], f32))
        anv = ctx.enter_context(nc.sbuf_tensor("anv", [128, MT], f32))
        pl_sb = ctx.enter_context(nc.sbuf_tensor("pl_sb", [128, 2 * MT], f32))
        ptall = ctx.enter_context(nc.psum_tensor("ptall", [128, 8, NCHUNK], f32))
        dsem = [ctx.enter_context(nc.semaphore(f"dsem{r}")) for r in range(NT)]
        dout = ctx.enter_context(nc.semaphore("dout"))
        mm_sem = ctx.enter_context(nc.semaphore("mm_sem"))
        asem = ctx.enter_context(nc.semaphore("asem"))
        fsem = ctx.enter_context(nc.semaphore("fsem"))
        wsem = ctx.enter_context(nc.semaphore("wsem"))
        isem = ctx.enter_context(nc.semaphore("isem"))
        done_sem = ctx.enter_context(nc.semaphore("done_sem"))
        block = ctx.enter_context(nc.Block())

        @block.gpsimd
        def _(gpsimd):
            nc.gpsimd.memset(wup[:], 0.0).then_inc(isem, 1)
            nc.gpsimd.memset(rmx[:], -FMAX).then_inc(isem, 1)

        @block.sync
        def _(sync):
            # region0 first (kp0/kp1 of round 0), lx second (kp2)
            sync.dma_start(rx3[:, 2:2 + KE, :],
                           rx[:, 0:KE * NCHUNK]).then_inc(dsem[0], 16)
            sync.dma_start(rx3[:, 0:2, :],
                           rx[:, KE * NCHUNK:(2 + KE) * NCHUNK]).then_inc(
                               dsem[0], 16)
            for r in range(1, NT):
                o = 2 * NCHUNK + r * SLOTW
                sync.dma_start(rx3[:, 2 + r * KE:2 + (r + 1) * KE, :],
                               rx[:, o:o + SLOTW]).then_inc(dsem[r], 16)
            sync.wait_ge(done_sem, 1)
            sync.dma_start(pl[:], pl_sb[:]).then_inc(dout, 16)
            sync.wait_ge(dout, 16)

        @block.tensor
        def _(tensor):
            tensor.wait_ge(isem, 1)
            for _ in range(N_WARMUP):
                nc.tensor.matmul(ptall[:, 7, 0:W_WARMUP], wup[:],
                                 wup[:, 0:W_WARMUP], start=True, stop=True)
            for r in range(NT):
                tensor.wait_ge(dsem[r], 16)
                for m in range(MT):
                    c = r * MT + m
                    b = bank(c)
                    if r >= 2:
                        # this tile's bank consumer from round r-2: the Act
                        # copy (m0-2) or the DVE direct fold (m3), plus any
                        # min-window reads of that tile
                        if m == 0:
                            tensor.wait_ge(asem, r - 1)
                        if m == MT - 1:
                            tensor.wait_ge(fsem, 2 * r - 3)
                        w = sum(1 for (_pm, pc, _a, _b) in pieces
                                if pc <= 4 * (r - 2) + m)
                        if w > 0:
                            tensor.wait_ge(wsem, w)
                    out = ptall[:, b, :]
                    ro = 2 + r * KE
                    for kp in range(KP - 1):
                        nc.tensor.matmul(
                            out,
                            rx3[:, 2 + 2 * kp:2 + 2 * kp + 2,
                                m * 128:(m + 1) * 128],
                            rx3[:, ro + 2 * kp:ro + 2 * kp + 2, :],
                            perf_mode=DR, start=(kp == 0), stop=False)
                    if r == 0 and m == 0:
                        tensor.wait_ge(dsem[0], 32)
                    mm = nc.tensor.matmul(
                        out,
                        rx3[:, 0:2, m * 128:(m + 1) * 128],
                        rx3[:, ro + 4:ro + 6, :],
                        perf_mode=DR, start=False, stop=True)
                    mm.then_inc(mm_sem, 1)

        @block.scalar
        def _(scalar):
            # stage tiles m0-2 of each round into SBUF bf16 for the DVE fold
            for r in range(NT):
                scalar.wait_ge(mm_sem, 4 * r + 3)
                if r >= 2:
                    scalar.wait_ge(fsem, 2 * (r - 2) + 2)
                h = (r % 2) * 4
                nc.scalar.copy(cbuf[r % 2][:],
                               ptall[:, h:h + 3, :]).then_inc(asem, 1)

        @block.vector
        def _(vector):
            vector.wait_ge(isem, 2)
            def emit_piece(i):
                m, c, a, b = pieces[i]
                vector.wait_ge(mm_sem, c + 1)
                bk = bank(c)
                nc.vector.tensor_reduce(
                    rw[:, i:i + 1], ptall[:, bk, a:b],
                    axis=ax.X, op=op.min).then_inc(wsem, 1)
            i = 0
            while i < n_pieces and pieces[i][1] < 8:
                emit_piece(i)
                i += 1
            for r in range(NT - 1):
                while i < n_pieces and pieces[i][1] + 1 <= 4 * r + 4:
                    emit_piece(i)
                    i += 1
                # direct f32 fold of tile m3 (gated only on the PE)
                vector.wait_ge(mm_sem, 4 * r + 4)
                h = (r % 2) * 4
                nc.vector.tensor_tensor(
                    rmx[:, MT - 1, :], ptall[:, h + 3, :],
                    rmx[:, MT - 1, :], op=op.max).then_inc(fsem, 1)
                # bf16 fold of the staged m0-2 tiles (2x DVE rate)
                vector.wait_ge(asem, r + 1)
                nc.vector.tensor_tensor(
                    rmx[:, 0:MT - 1, :], cbuf[r % 2][:], rmx[:, 0:MT - 1, :],
                    op=op.max).then_inc(fsem, 1)
                if r == 3:
                    # min-side combines: all window pieces are in by round 3
                    # and DVE has slack here, keeping them off the tail
                    nc.vector.drain()
                    for m in range(MT):
                        idx = [j for j, p in enumerate(pieces) if p[0] == m]
                        if len(idx) == 1:
                            nc.vector.tensor_scalar(
                                pl_sb[:, MT + m:MT + m + 1],
                                rw[:, idx[0]:idx[0] + 1], FMAX, 0.0,
                                op0=op.min)
                        else:
                            nc.vector.tensor_tensor(
                                pl_sb[:, MT + m:MT + m + 1],
                                rw[:, idx[0]:idx[0] + 1],
                                rw[:, idx[1]:idx[1] + 1], op=op.min)
            # round 7: fold m3 directly, fold the staged m0-2, then one
            # reduce over the whole running max
            rl = NT - 1
            vector.wait_ge(mm_sem, 4 * rl + 4)
            h = (rl % 2) * 4
            nc.vector.tensor_tensor(
                rmx[:, MT - 1, :], ptall[:, h + 3, :],
                rmx[:, MT - 1, :], op=op.max)
            vector.wait_ge(asem, rl + 1)
            nc.vector.tensor_tensor(
                rmx[:, 0:MT - 1, :], cbuf[rl % 2][:], rmx[:, 0:MT - 1, :],
                op=op.max)
            fin = nc.vector.tensor_reduce(
                pl_sb[:, 0:MT], rmx[:], axis=ax.X, op=op.max)
            fin.then_inc(done_sem, 1)
    return nc


def _prep(feat: np.ndarray, targets: np.ndarray):
    import ml_dtypes
    fp8 = ml_dtypes.float8_e4m3
    feat = np.asarray(feat, dtype=np.float32)
    tg = np.asarray(targets).astype(np.int64)

    perm = np.argsort(tg, kind="stable")
    tgs = tg[perm]
    feats = feat[perm]

    counts = np.bincount(tgs, minlength=NCLS)
    # window margin: a block's first/last class can extend at most
    # count-1 columns beyond the block boundary
    mar = max(32, int(counts.max()) - 1)
    assert mar <= 384, "class sizes too skewed for the window scheme"

    featx = np.zeros((KE * 128, N_ROWS), dtype=fp8)
    featx[:D, :] = feats.T.astype(fp8)
    featx[D + tgs, np.arange(N_ROWS)] = fp8(-CODE)

    in_maps = []
    for c in range(N_CORES):
        rxa = np.empty((128, 2 * NCHUNK + NT * SLOTW), dtype=fp8)
        tloc = tgs[c * M:(c + 1) * M]
        lxa = np.zeros((128, 2 * NCHUNK), dtype=fp8)
        lxa[tloc, np.arange(M)] = fp8(CODE)
        # layout: [region0 | lx | region1..7] (region0 DMAs first)
        rxa[:, KE * NCHUNK:(2 + KE) * NCHUNK] = lxa
        for r in range(NT):
            gc = (c - 1 + L[r]) % NT
            blk = featx[:, gc * NCHUNK:(gc + 1) * NCHUNK]
            dst = 0 if r == 0 else 2 * NCHUNK + r * SLOTW
            rxa[:, dst:dst + SLOTW] = (
                blk.reshape(KE, 128, NCHUNK).transpose(1, 0, 2).reshape(128, SLOTW))
        in_maps.append({"rx": rxa})
    return in_maps, mar


def kernel(feat: np.ndarray, targets: np.ndarray) -> np.ndarray:
    in_maps, mar = _prep(feat, targets)
    key = ("nc", mar)
    if key not in _CACHE:
        _CACHE[key] = _build(mar)
    nc = _CACHE[key]
    res = run_bass_kernel_spmd(nc, in_maps, list(range(N_CORES)))
    total = 0.0
    for c in range(N_CORES):
        out = res.results[c]["pl"].astype(np.float64)
        an = out[:, :MT]
        ap = out[:, MT:]
        total += np.maximum(an - ap + MARGIN - BIG, 0.0).sum()
    return np.asarray(np.float32(total / N_ROWS))


# revision 8
# speedup vs baseline: 1.8449x; 1.0164x over previous
"""Trainium2 Bass kernel for nn_CorrLoss — v5.

fp8e4 DoubleRow Gram matmuls (3 k-pair steps per [128,512] tile) with the
one-hot class-code contraction extension (corr' = corr - 16384*same).
Rows+columns class-sorted on host: the min-over-positives side is a few
narrow exact f32 PSUM window reduces; the max-over-negatives side is a
running elementwise max: per round, tile m3 folds straight from PSUM f32
on DVE while tiles m0-2 are staged to SBUF as bf16 by the Act engine and
folded at 2x DVE rate.  One [128,4,512] reduce finishes the max side.
Row-data-parallel across 8 cores via rotated column-chunk upload.
"""
import sys
from contextlib import ExitStack

import numpy as np

sys.path.insert(0, "/opt/trn_rl_repo")

import concourse.bass as bass  # noqa: E402
from concourse import mybir  # noqa: E402
from concourse.bass_utils import run_bass_kernel_spmd  # noqa: E402

N_CORES = 8
N_ROWS = 4096
D = 512
NCLS = 64
M = N_ROWS // N_CORES      # 512 local rows
MT = M // 128              # 4 row tiles
NCHUNK = 512
NT = N_ROWS // NCHUNK      # 8 column chunks
KE = 6                     # 4 feature k-blocks + one-hot ext + zero pad
KP = KE // 2               # 3 DoubleRow pairs
SLOTW = KE * NCHUNK        # 3072
MARGIN = 40.0
BIG = 16384.0
CODE = 128.0
N_WARMUP = 48
W_WARMUP = 64
L = [1, 0, 2, 3, 4, 5, 6, 7]
POS_OF_SLOT = {1: 0, 0: 1, 2: 2}

_CACHE = {}


def _window_pieces(mar):
    pieces = []
    for m in range(MT):
        lo = 512 + 128 * m - mar
        hi = 512 + 128 * m + 128 + mar
        for s in (0, 1, 2):
            a = max(lo, 512 * s)
            b = min(hi, 512 * s + 512)
            if a < b:
                c = 4 * POS_OF_SLOT[s] + m
                pieces.append((m, c, a - 512 * s, b - 512 * s))
    pieces.sort(key=lambda p: p[1])
    return pieces


def _build(mar):
    f32 = mybir.dt.float32
    bf16 = mybir.dt.bfloat16
    fp8 = mybir.dt.float8e4
    op = mybir.AluOpType
    ax = mybir.AxisListType
    DR = mybir.MatmulPerfMode.DoubleRow
    FMAX = 3.4e38
    pieces = _window_pieces(mar)
    n_pieces = len(pieces)

    def wsem_need(r):
        return sum(1 for (_m, c, _a, _b) in pieces if c < 4 * (r - 1))

    nc = bass.Bass("TRN2", target_bir_lowering=False, debug=False)
    # host layout: [lx(ext,pad: 1024) | region0 (3072) | ... | region7]
    rx = nc.declare_dram_parameter("rx", [128, 2 * NCHUNK + NT * SLOTW], fp8,
                                   isOutput=False)
    pl = nc.declare_dram_parameter("pl", [128, 2 * MT], f32, isOutput=True)

    def bank(c):
        return ((c // 4) % 2) * 4 + (c % 4)

    with ExitStack() as ctx:
        # dim1 k-blocks: 0-1 = lx (+code ext, zero pad); region r at 2+6r
        rx3 = ctx.enter_context(
            nc.sbuf_tensor("rx3", [128, 2 + NT * KE, NCHUNK], fp8))
        wup = ctx.enter_context(nc.sbuf_tensor("wup", [128, 128], fp8))
        cbuf = [ctx.enter_context(nc.sbuf_tensor(f"cbuf{i}", [128, 3, NCHUNK], bf16))
                for i in range(2)]
        rmx = ctx.enter_context(
            nc.sbuf_tensor("rmx", [128, MT - 1, NCHUNK], bf16))
        t3acc = ctx.enter_context(nc.sbuf_tensor("t3acc", [128, NT], f32))
        rw = ctx.enter_context(nc.sbuf_tensor("rw", [128, n_pieces], f32))
        an6 = ctx.enter_context(nc.sbuf_tensor("an6", [128, MT], f32))
        apv = ctx.enter_context(nc.sbuf_tensor("apv", [128, MT], f32))
        anv = ctx.enter_context(nc.sbuf_tensor("anv", [128, MT], f32))
        pl_sb = ctx.enter_context(nc.sbuf_tensor("pl_sb", [128, 2 * MT], f32))
        ptall = ctx.enter_context(nc.psum_tensor("ptall", [128, 8, NCHUNK], f32))
        dsem = [ctx.enter_context(nc.semaphore(f"dsem{r}")) for r in range(NT)]
        dout = ctx.enter_context(nc.semaphore("dout"))
        mm_sem = ctx.enter_context(nc.semaphore("mm_sem"))
        asem = ctx.enter_context(nc.semaphore("asem"))
        fsem = ctx.enter_context(nc.semaphore("fsem"))
        wsem = ctx.enter_context(nc.semaphore("wsem"))
        isem = ctx.enter_context(nc.semaphore("isem"))
        done_sem = ctx.enter_context(nc.semaphore("done_sem"))
        block = ctx.enter_context(nc.Block())

        @block.gpsimd
        def _(gpsimd):
            nc.gpsimd.memset(wup[:], 0.0).then_inc(isem, 1)
            nc.gpsimd.memset(rmx[:], -FMAX)
            nc.gpsimd.memset(t3acc[:], -FMAX).then_inc(isem, 1)

        @block.sync
        def _(sync):
            # region0 first (kp0/kp1 of round 0), lx second (kp2)
            sync.dma_start(rx3[:, 2:2 + KE, :],
                           rx[:, 0:KE * NCHUNK]).then_inc(dsem[0], 16)
            sync.dma_start(rx3[:, 0:2, :],
                           rx[:, KE * NCHUNK:(2 + KE) * NCHUNK]).then_inc(
                               dsem[0], 16)
            for r in range(1, NT):
                o = 2 * NCHUNK + r * SLOTW
                sync.dma_start(rx3[:, 2 + r * KE:2 + (r + 1) * KE, :],
                               rx[:, o:o + SLOTW]).then_inc(dsem[r], 16)
            sync.wait_ge(done_sem, 1)
            sync.dma_start(pl[:], pl_sb[:]).then_inc(dout, 16)
            sync.wait_ge(dout, 16)

        @block.tensor
        def _(tensor):
            tensor.wait_ge(isem, 1)
            for _ in range(N_WARMUP):
                nc.tensor.matmul(ptall[:, 7, 0:W_WARMUP], wup[:],
                                 wup[:, 0:W_WARMUP], start=True, stop=True)
            for r in range(NT):
                tensor.wait_ge(dsem[r], 16)
                for m in range(MT):
                    c = r * MT + m
                    b = bank(c)
                    if r >= 2:
                        # this tile's bank consumer from round r-2: the Act
                        # copy (m0-2) or the DVE direct fold (m3), plus any
                        # min-window reads of that tile
                        if m == 0:
                            tensor.wait_ge(asem, r - 1)
                        if m == MT - 1:
                            tensor.wait_ge(fsem, 2 * r - 3)
                        w = sum(1 for (_pm, pc, _a, _b) in pieces
                                if pc <= 4 * (r - 2) + m)
                        if w > 0:
                            tensor.wait_ge(wsem, w)
                    out = ptall[:, b, :]
                    ro = 2 + r * KE
                    for kp in range(KP - 1):
                        nc.tensor.matmul(
                            out,
                            rx3[:, 2 + 2 * kp:2 + 2 * kp + 2,
                                m * 128:(m + 1) * 128],
                            rx3[:, ro + 2 * kp:ro + 2 * kp + 2, :],
                            perf_mode=DR, start=(kp == 0), stop=False)
                    if r == 0 and m == 0:
                        tensor.wait_ge(dsem[0], 32)
                    mm = nc.tensor.matmul(
                        out,
                        rx3[:, 0:2, m * 128:(m + 1) * 128],
                        rx3[:, ro + 4:ro + 6, :],
                        perf_mode=DR, start=False, stop=True)
                    mm.then_inc(mm_sem, 1)

        @block.scalar
        def _(scalar):
            # stage tiles m0-2 of each round into SBUF bf16 for the DVE fold
            for r in range(NT):
                scalar.wait_ge(mm_sem, 4 * r + 3)
                if r >= 2:
                    scalar.wait_ge(fsem, 2 * (r - 2) + 2)
                h = (r % 2) * 4
                nc.scalar.copy(cbuf[r % 2][:],
                               ptall[:, h:h + 3, :]).then_inc(asem, 1)

        @block.vector
        def _(vector):
            vector.wait_ge(isem, 2)
            def emit_piece(i):
                m, c, a, b = pieces[i]
                vector.wait_ge(mm_sem, c + 1)
                bk = bank(c)
                nc.vector.tensor_reduce(
                    rw[:, i:i + 1], ptall[:, bk, a:b],
                    axis=ax.X, op=op.min).then_inc(wsem, 1)
            i = 0
            while i < n_pieces and pieces[i][1] < 8:
                emit_piece(i)
                i += 1
            for r in range(NT - 1):
                while i < n_pieces and pieces[i][1] + 1 <= 4 * r + 4:
                    emit_piece(i)
                    i += 1
                # direct f32 reduce of tile m3 (gated only on the PE):
                # cheaper than folding, since m3 then skips the final reduce
                vector.wait_ge(mm_sem, 4 * r + 4)
                h = (r % 2) * 4
                nc.vector.tensor_reduce(
                    t3acc[:, r:r + 1], ptall[:, h + 3, :],
                    axis=ax.X, op=op.max).then_inc(fsem, 1)
                # bf16 fold of the staged m0-2 tiles (2x DVE rate)
                vector.wait_ge(asem, r + 1)
                nc.vector.tensor_tensor(
                    rmx[:], cbuf[r % 2][:], rmx[:],
                    op=op.max).then_inc(fsem, 1)
                if r == 3:
                    # min-side combines: all window pieces are in by round 3
                    # and DVE has slack here, keeping them off the tail
                    nc.vector.drain()
                    for m in range(MT):
                        idx = [j for j, p in enumerate(pieces) if p[0] == m]
                        if len(idx) == 1:
                            nc.vector.tensor_scalar(
                                pl_sb[:, MT + m:MT + m + 1],
                                rw[:, idx[0]:idx[0] + 1], FMAX, 0.0,
                                op0=op.min)
                        else:
                            nc.vector.tensor_tensor(
                                pl_sb[:, MT + m:MT + m + 1],
                                rw[:, idx[0]:idx[0] + 1],
                                rw[:, idx[1]:idx[1] + 1], op=op.min)
            # round 7: direct reduce of m3, fold the staged m0-2, reduce
            # the m0-2 running max, then combine the per-round m3 maxima
            rl = NT - 1
            vector.wait_ge(mm_sem, 4 * rl + 4)
            h = (rl % 2) * 4
            nc.vector.tensor_reduce(
                t3acc[:, rl:rl + 1], ptall[:, h + 3, :], axis=ax.X, op=op.max)
            vector.wait_ge(asem, rl + 1)
            nc.vector.tensor_tensor(
                rmx[:], cbuf[rl % 2][:], rmx[:], op=op.max)
            nc.vector.drain()
            nc.vector.tensor_reduce(
                pl_sb[:, 0:MT - 1], rmx[:], axis=ax.X, op=op.max)
            fin = nc.vector.tensor_reduce(
                pl_sb[:, MT - 1:MT], t3acc[:], axis=ax.X, op=op.max)
            fin.then_inc(done_sem, 1)
    return nc


def _prep(feat: np.ndarray, targets: np.ndarray):
    import ml_dtypes
    fp8 = ml_dtypes.float8_e4m3
    feat = np.asarray(feat, dtype=np.float32)
    tg = np.asarray(targets).astype(np.int64)

    perm = np.argsort(tg, kind="stable")
    tgs = tg[perm]
    feats = feat[perm]

    counts = np.bincount(tgs, minlength=NCLS)
    # window margin: a block's first/last class can extend at most
    # count-1 columns beyond the block boundary
    mar = max(32, int(counts.max()) - 1)
    assert mar <= 384, "class sizes too skewed for the window scheme"

    featx = np.zeros((KE * 128, N_ROWS), dtype=fp8)
    featx[:D, :] = feats.T.astype(fp8)
    featx[D + tgs, np.arange(N_ROWS)] = fp8(-CODE)

    in_maps = []
    for c in range(N_CORES):
        rxa = np.empty((128, 2 * NCHUNK + NT * SLOTW), dtype=fp8)
        tloc = tgs[c * M:(c + 1) * M]
        lxa = np.zeros((128, 2 * NCHUNK), dtype=fp8)
        lxa[tloc, np.arange(M)] = fp8(CODE)
        # layout: [region0 | lx | region1..7] (region0 DMAs first)
        rxa[:, KE * NCHUNK:(2 + KE) * NCHUNK] = lxa
        for r in range(NT):
            gc = (c - 1 + L[r]) % NT
            blk = featx[:, gc * NCHUNK:(gc + 1) * NCHUNK]
            dst = 0 if r == 0 else 2 * NCHUNK + r * SLOTW
            rxa[:, dst:dst + SLOTW] = (
                blk.reshape(KE, 128, NCHUNK).transpose(1, 0, 2).reshape(128, SLOTW))
        in_maps.append({"rx": rxa})
    return in_maps, mar


def kernel(feat: np.ndarray, targets: np.ndarray) -> np.ndarray:
    in_maps, mar = _prep(feat, targets)
    key = ("nc", mar)
    if key not in _CACHE:
        _CACHE[key] = _build(mar)
    nc = _CACHE[key]
    res = run_bass_kernel_spmd(nc, in_maps, list(range(N_CORES)))
    total = 0.0
    for c in range(N_CORES):
        out = res.results[c]["pl"].astype(np.float64)
        an = out[:, :MT]
        ap = out[:, MT:]
        total += np.maximum(an - ap + MARGIN - BIG, 0.0).sum()
    return np.asarray(np.float32(total / N_ROWS))
